# revision 46
# baseline (speedup 1.0000x reference)
"""Trainium2 Bass kernel for the DecoderSVM SNN decoder.

reference computation:
    curr[t,b,o] = einsum('bit,oi->tbo', inputs, W) + b         (I=182 -> O=2)
    syn_t = clip(alpha,0,1)*syn_{t-1} + curr_t                 (scan over T)
    mem_t = clip(beta,0,1)*mem_{t-1} + syn_t
    out = mem_rec transposed to [B, T, O]

Strategy (8 NeuronCores, batch-sharded 32 per core), fp8 DoubleRow edition:
  - Inputs are centered (x - 0.5) and cast to fp8_e4m3; the exact mean
    term 0.5*sum_i W[o,i] + b[o] is folded into a bias constant host-side
    (in f32), so fp8's coarse mantissa only touches the zero-mean part.
    Measured end-to-end rel err ~1.04e-2 vs the 2e-2 gate.
  - fp8 halves HBM traffic (11.65 MB/core) and DoubleRow matmul
    (perf_mode, 2 fp8 MACs/partition/cycle, K-tiles of 2) nearly halves PE
    time: 8 input rows per chunk -> 22 full chunks + 1 tail chunk.
  - Block-diagonal stationary lhsT [128, 2, 64-wide W block]: K = 32
    batches x 4 partition-rows (x 2 k-tiles), 64 (batch, o) outputs.
  - The bias constant rides in the tail chunk as two extra K partitions
    (96: hi, 97: lo in fp8) against host-baked ones rows.
  - Time axis split in half across PSUM partitions: partitions 0-63 hold
    t in [0,1000), partitions 64-127 hold t in [1000,2000).  The ISA
    rejects DoubleRow matmuls at a column tile offset, so every matmul is
    full-PE [128, w] with a zero-padded 128-wide weight window: the
    stationary buffer interleaves 64-wide W blocks with 64-wide zero gaps
    ([z64, W0, z64, W1, ...], stride 128), and half h of chunk c slices
    window [128c + 64(1-h) : +128].  The wrong half accumulates zeros.
  - Both halves scan in parallel in single tensor_tensor_scan calls (the
    scan is the serial tail; this halves it).  The half-boundary carry is
    fixed up at the end: syn_999 is copied and mem_999 estimated from the
    last 24 syn columns (beta^24 < 1e-6) right after the syn scans, so
    one tiny partition-shift DMA overlaps the mem scans, and
    mem[1000..1063] += G1*syn_999 + G2*mem_999 with host-precomputed
    geometric tables lands right after the last scan.
  - DMA: x groups alternate the sync/scalar HWDGE queues strictly (each
    HWDGE engine only keeps ~5 DMAs in flight, so <=5 early triggers per
    queue), sizes tuned so both queues carry equal bytes and completion
    order matches program order; the alpha/beta scan broadcast is built
    on-chip by the (idle) VectorE instead of DMAing 0.5 MB.
"""

import numpy as np
import ml_dtypes

B, I, T, O = 256, 182, 2000, 2
NCORES = 8
NB = B // NCORES              # 32 batches per core
M = 2 * NB                    # 64 (batch, o) pairs per time-half
TH = T // 2                   # 1000 time steps per half
NCH = 22                      # full DoubleRow chunks of 8 rows (176 rows)
KTAIL = 3 * NB + 2            # 96 data partitions + 2 bias partitions
LWCOL = 64 + 128 * NCH        # shared-zero-gap stationary layout length
GROUPS = [2, 3, 4, 4, 4, 3, 2]   # chunks per DMA group (sum = NCH)
GQUEUE = [0, 1, 0, 1, 0, 1, 0]   # 0=sync, 1=scalar; strict alternation
TSPLIT = [512, 488]              # PSUM-bank time tiles per half
NCORR = 64                    # carry-correction columns (decay < 1e-7)
NEST = 24                     # syn columns used to estimate mem_999

FP8 = ml_dtypes.float8_e4m3   # TRN FP8_EXP4 (max +-240)

TRACE = False

_cache = {}


def _build_nc():
    import concourse.bacc as bacc
    import concourse.bass as bass
    import concourse.mybir as mybir
    from concourse.tile import TileContext

    f32 = mybir.dt.float32
    fp8 = mybir.dt.float8e4
    DR = mybir.MatmulPerfMode.DoubleRow
    mult, add = mybir.AluOpType.mult, mybir.AluOpType.add

    nc = bacc.Bacc("TRN2", target_bir_lowering=False, debug=False)

    x = nc.dram_tensor("x", [NB, 8 * NCH, T], fp8, kind="ExternalInput")
    x_tail = nc.dram_tensor("x_tail", [KTAIL, 2, T], fp8, kind="ExternalInput")
    lhsT_full = nc.dram_tensor("lhsT_full", [128, 2, LWCOL], fp8, kind="ExternalInput")
    lhsT_tail = nc.dram_tensor("lhsT_tail", [KTAIL, 2, 3 * M], fp8, kind="ExternalInput")
    # packed f32 consts: [:,0,:] G1, [:,1,:] G2 (partitions 64-127);
    # [0:64,2,0:NEST] w24 estimate weights; [:,3,0:2] alpha,beta
    sconst = nc.dram_tensor("sconst", [128, 4, NCORR], f32, kind="ExternalInput")
    y = nc.dram_tensor("y", [M, T], mybir.dt.bfloat16, kind="ExternalOutput")

    with TileContext(nc) as tc:
        with (
            tc.tile_pool(name="consts", bufs=1) as cpool,
            tc.tile_pool(name="xs", bufs=6) as xpool,
            tc.tile_pool(name="xl", bufs=1) as xlpool,
            tc.tile_pool(name="mems", bufs=1) as mpool,
            tc.tile_pool(name="psum", bufs=1, space=bass.MemorySpace.PSUM) as ppool,
        ):
            sco = cpool.tile([128, 4, NCORR], f32)
            nc.scalar.dma_start(out=sco[:], in_=sconst[:])
            lw = cpool.tile([128, 2, LWCOL], fp8)
            nc.scalar.dma_start(out=lw[:], in_=lhsT_full[:])
            lwt = cpool.tile([KTAIL, 2, 3 * M], fp8)
            nc.sync.dma_start(out=lwt[:], in_=lhsT_tail[:])

            # alpha/beta scan broadcasts built on-chip (VectorE is idle
            # during the load phase): abbb[:,k,:] = 0 + sconst[:,3,k]
            ztile = cpool.tile([128, 512], f32)
            nc.vector.memset(ztile[:], 0.0)
            abbb = cpool.tile([128, 2, 512], f32)
            for k in range(2):
                nc.vector.scalar_tensor_tensor(
                    out=abbb[:, k, :],
                    in0=ztile[:],
                    scalar=sco[:, 3, k : k + 1],
                    in1=ztile[:],
                    op0=add,
                    op1=add,
                )

            # one PSUM tile per time-tile: separate tensors so the tile-0
            # scans don't get a false dependency on the PE's tile-1 writes
            pt0 = ppool.tile([128, 512], f32)
            pt1 = ppool.tile([128, 512], f32)
            ptf = ppool.tile([128, 64], f32)
            qs = [nc.sync, nc.scalar]

            def chunk_matmuls(wsel, rhs3, c, tiles):
                """wsel(h) -> [K, 2, 128] stationary window for half h;
                rhs3: [K, 2, T] moving data; one matmul per (tile, half)."""
                for ti in tiles:
                    off = 512 * ti
                    w = TSPLIT[ti]
                    for h in range(2):
                        t0 = TH * h + off
                        nc.tensor.matmul(
                            (pt0, pt1)[ti][:, 0:w],
                            wsel(h),
                            rhs3[:, :, t0 : t0 + w],
                            start=(c == 0 and h == 0),
                            stop=(c == NCH and h == 1),
                            perf_mode=DR,
                        )

            c0 = 0
            for gi, G in enumerate(GROUPS):
                r0 = 8 * c0
                xt = xpool.tile([128, 2 * G, T], fp8, tag="xt")
                src = x[:, r0 : r0 + 8 * G, :].rearrange(
                    "b (i r) t -> b i r t", i=4, r=2 * G
                )
                qs[GQUEUE[gi]].dma_start(out=xt[:], in_=src)
                if gi == 5:
                    # tail chunk data (rows 176..181 regrouped + baked ones
                    # rows for the bias) -- early in the scalar FIFO so the
                    # final accumulation isn't gated on it
                    xe = xlpool.tile([KTAIL, 2, T], fp8)
                    nc.scalar.dma_start(out=xe[:], in_=x_tail[:])
                last = gi == len(GROUPS) - 1

                def wfull(c):
                    return lambda h: lw[:, :, 128 * c + 64 * (1 - h) : 128 * c + 64 * (1 - h) + 128]

                def wtail(h):
                    return lwt[:, :, 64 * (1 - h) : 64 * (1 - h) + 128]

                if gi >= len(GROUPS) - 2:
                    # tiny self-contained filler matmuls: they run during the
                    # DMA wait for this group and keep the PE's HAM clock
                    # gate at K=8/8 (2.4 GHz) for the post-DMA tail burst
                    for _ in range(24):
                        nc.tensor.matmul(
                            ptf[:, 0:64],
                            lw[:, :, 0:128],
                            lw[:, 0:2, 0:64],
                            start=True,
                            stop=True,
                            perf_mode=DR,
                        )
                if not last:
                    for cc in range(G):
                        chunk_matmuls(
                            wfull(c0 + cc),
                            xt[:, 2 * cc : 2 * cc + 2, :],
                            c0 + cc,
                            (0, 1),
                        )
                else:
                    # tile-major for the last group + tail chunk so tile 0's
                    # accumulation closes (and scanning starts) ASAP
                    for ti in range(2):
                        for cc in range(G):
                            chunk_matmuls(
                                wfull(c0 + cc),
                                xt[:, 2 * cc : 2 * cc + 2, :],
                                c0 + cc,
                                (ti,),
                            )
                        chunk_matmuls(wtail, xe[:], NCH, (ti,))
                c0 += G

            syn = mpool.tile([128, TH], f32)
            mem = mpool.tile([128, TH], mybir.dt.bfloat16)
            carry = mpool.tile([128, 2], f32)
            st2 = mpool.tile([128, 2], f32)
            tmp24 = mpool.tile([128, NEST], f32)
            tmp1 = mpool.tile([128, NCORR], f32)

            def scan(out, k, d1, ti):
                off = 512 * ti
                w = TSPLIT[ti]
                nc.vector.tensor_tensor_scan(
                    out[:, off : off + w],
                    abbb[:, k, :w],
                    d1,
                    initial=(0.0 if ti == 0 else out[:, off - 1 : off]),
                    op0=mult,
                    op1=add,
                )

            scan(syn, 0, pt0[:, 0:512], 0)
            scan(syn, 0, pt1[:, 0:488], 1)
            # half-boundary carries, computed from syn alone right after the
            # syn scans (before BOTH mem scans, so the tiny partition-shift
            # DMA fully hides under them): copy syn_999 and estimate
            # mem_999 = sum of the last NEST syn columns * beta^(999-s)
            nc.vector.scalar_tensor_tensor(
                out=st2[0:64, 0:1],
                in0=syn[0:64, TH - 1 : TH],
                scalar=0.0,
                in1=ztile[0:64, 0:1],
                op0=add,
                op1=add,
            )
            nc.vector.scalar_tensor_tensor(
                out=tmp24[0:64, :],
                in0=syn[0:64, TH - NEST : TH],
                scalar=1.0,
                in1=sco[0:64, 2, 0:NEST],
                op0=mult,
                op1=mult,
                accum_out=st2[0:64, 1:2],
            )
            nc.sync.dma_start(out=carry[64:128, 0:2], in_=st2[0:64, 0:2])
            scan(mem, 1, syn[:, 0:512], 0)
            # first-half early outputs (everything not behind the carry corr)
            nc.sync.dma_start(out=y[:, :512], in_=mem[0:64, :512])
            nc.scalar.dma_start(
                out=y[:, TH + NCORR : TH + 512], in_=mem[64:128, NCORR:512]
            )
            scan(mem, 1, syn[:, 512:1000], 1)
            # mem[1000+j] += G1[j]*syn_999 + G2[j]*mem_999  (j < NCORR)
            nc.vector.scalar_tensor_tensor(
                out=tmp1[64:128, :],
                in0=sco[64:128, 0, :],
                scalar=carry[64:128, 0:1],
                in1=mem[64:128, :NCORR],
                op0=mult,
                op1=add,
            )
            nc.vector.scalar_tensor_tensor(
                out=mem[64:128, :NCORR],
                in0=sco[64:128, 1, :],
                scalar=carry[64:128, 1:2],
                in1=tmp1[64:128, :],
                op0=mult,
                op1=add,
            )
            # remaining outputs: second tiles of each half, then the NCORR
            # corrected columns last (tiny post-correction tail)
            nc.sync.dma_start(out=y[:, 512:TH], in_=mem[0:64, 512:TH])
            nc.scalar.dma_start(out=y[:, TH + 512 : T], in_=mem[64:128, 512:TH])
            nc.scalar.dma_start(out=y[:, TH : TH + NCORR], in_=mem[64:128, :NCORR])

    nc.compile()
    return nc


def _host_tensors(W, b, alpha, beta):
    """Stationary fp8 weights (shared-zero-gap layout) + packed consts."""
    W = np.asarray(W, np.float32)
    bvec = np.asarray(b, np.float32)
    a_cl = np.clip(np.asarray(alpha, np.float32), 0.0, 1.0)
    bt_cl = np.clip(np.asarray(beta, np.float32), 0.0, 1.0)

    W8 = W.astype(FP8).astype(np.float32)
    bias_fold = (
        bvec.astype(np.float64) + 0.5 * W.astype(np.float64).sum(axis=1)
    ).astype(np.float32)
    bias_hi = bias_fold.astype(FP8).astype(np.float32)
    bias_lo = (bias_fold - bias_hi).astype(FP8).astype(np.float32)

    bidx = np.arange(NB)
    # [z64, W0, z64, W1, ..., z64] with 128 stride between W blocks; the
    # 64-wide zero gaps are shared between half-windows of adjacent chunks
    lhsT = np.zeros((128, 2, LWCOL), np.float32)
    c0 = 0
    for G in GROUPS:
        r0 = 8 * c0
        for cc in range(G):
            c = c0 + cc
            for i in range(4):
                for k in range(2):
                    row = r0 + 2 * G * i + 2 * cc + k
                    for o in range(O):
                        lhsT[4 * bidx + i, k, 128 * c + 64 + 2 * bidx + o] = W8[o, row]
        c0 += G
    assert c0 == NCH

    lhsT_tail = np.zeros((KTAIL, 2, 3 * M), np.float32)
    for i in range(3):
        for k in range(2):
            row = 8 * NCH + 2 * i + k
            for o in range(O):
                lhsT_tail[3 * bidx + i, k, M + 2 * bidx + o] = W8[o, row]
    for o in range(O):
        lhsT_tail[96, 0, M + 2 * bidx + o] = bias_hi[o]
        lhsT_tail[96, 1, M + 2 * bidx + o] = bias_lo[o]

    # packed consts: G1[j] = sum_{s<=j} beta^(j-s)*alpha^(s+1), G2[j] =
    # beta^(j+1) (carry corrections), w24[j] = beta^(NEST-1-j) (mem_999
    # estimate), alpha/beta per partition
    sconst = np.zeros((128, 4, NCORR), np.float32)
    for o in range(O):
        a_, b_ = float(a_cl[o]), float(bt_cl[o])
        g1 = np.empty(NCORR, np.float64)
        acc = 0.0
        apow = 1.0
        for j in range(NCORR):
            apow *= a_
            acc = b_ * acc + apow
            g1[j] = acc
        sconst[64 + o :: 2, 0, :][:] = g1.astype(np.float32)[None, :][
            :, : NCORR
        ]
        sconst[64 + o :: 2, 1, :] = (
            b_ ** np.arange(1, NCORR + 1, dtype=np.float64)
        ).astype(np.float32)
        sconst[o:64:2, 2, 0:NEST] = (
            b_ ** np.arange(NEST - 1, -1, -1, dtype=np.float64)
        ).astype(np.float32)
        sconst[o::2, 3, 0] = a_
        sconst[o::2, 3, 1] = b_

    return lhsT.astype(FP8), lhsT_tail.astype(FP8), np.ascontiguousarray(sconst)


def kernel(inputs, W, b, alpha, beta):
    from concourse.bass_utils import run_bass_kernel_spmd

    if "nc" not in _cache:
        _cache["nc"] = _build_nc()
    nc = _cache["nc"]

    lhsT_full, lhsT_tail, sconst = _host_tensors(W, b, alpha, beta)

    x_c = (np.asarray(inputs, np.float32) - np.float32(0.5)).astype(FP8)  # [B, I, T]

    in_maps = []
    for c in range(NCORES):
        xc = x_c[c * NB : (c + 1) * NB]
        # tail rows 176..182 regrouped to [96, 2, T] + two ones rows for bias
        xt = np.empty((KTAIL, 2, T), FP8)
        xt[:96] = xc[:, 176:182, :].reshape(NB * 3, 2, T)
        xt[96:] = np.float32(1.0)
        in_maps.append(
            {
                "x": np.ascontiguousarray(xc[:, : 8 * NCH]),
                "x_tail": xt,
                "lhsT_full": lhsT_full,
                "lhsT_tail": lhsT_tail,
                "sconst": sconst,
            }
        )

    res = run_bass_kernel_spmd(nc, in_maps, core_ids=list(range(NCORES)), trace=TRACE)
    kernel.last_exec_time_ns = res.exec_time_ns
    kernel.last_result = res
    out = np.empty((B, O, T), np.float32)
    for c in range(NCORES):
        out[c * NB : (c + 1) * NB] = res.results[c]["y"].astype(np.float32).reshape(
            NB, O, T
        )
    return np.ascontiguousarray(out.transpose(0, 2, 1))


kernel.last_exec_time_ns = None
kernel.last_result = None


# revision 47
# speedup vs baseline: 1.0126x; 1.0126x over previous
"""Trainium2 Bass kernel for the DecoderSVM SNN decoder.

reference computation:
    curr[t,b,o] = einsum('bit,oi->tbo', inputs, W) + b         (I=182 -> O=2)
    syn_t = clip(alpha,0,1)*syn_{t-1} + curr_t                 (scan over T)
    mem_t = clip(beta,0,1)*mem_{t-1} + syn_t
    out = mem_rec transposed to [B, T, O]

Strategy (8 NeuronCores, batch-sharded 32 per core), fp8 DoubleRow edition:
  - Inputs are centered (x - 0.5) and cast to fp8_e4m3; the exact mean
    term 0.5*sum_i W[o,i] + b[o] is folded into a bias constant host-side
    (in f32), so fp8's coarse mantissa only touches the zero-mean part.
    Measured end-to-end rel err ~1.04e-2 vs the 2e-2 gate.
  - fp8 halves HBM traffic (11.65 MB/core) and DoubleRow matmul
    (perf_mode, 2 fp8 MACs/partition/cycle, K-tiles of 2) nearly halves PE
    time: 8 input rows per chunk -> 22 full chunks + 1 tail chunk.
  - Block-diagonal stationary lhsT [128, 2, 64-wide W block]: K = 32
    batches x 4 partition-rows (x 2 k-tiles), 64 (batch, o) outputs.
  - The bias constant rides in the tail chunk as two extra K partitions
    (96: hi, 97: lo in fp8) against host-baked ones rows.
  - Time axis split in half across PSUM partitions: partitions 0-63 hold
    t in [0,1000), partitions 64-127 hold t in [1000,2000).  The ISA
    rejects DoubleRow matmuls at a column tile offset, so every matmul is
    full-PE [128, w] with a zero-padded 128-wide weight window: the
    stationary buffer interleaves 64-wide W blocks with 64-wide zero gaps
    ([z64, W0, z64, W1, ...], stride 128), and half h of chunk c slices
    window [128c + 64(1-h) : +128].  The wrong half accumulates zeros.
  - Both halves scan in parallel in single tensor_tensor_scan calls (the
    scan is the serial tail; this halves it).  The half-boundary carry is
    fixed up at the end: syn_999 is copied and mem_999 estimated from the
    last 24 syn columns (beta^24 < 1e-6) right after the syn scans, so
    one tiny partition-shift DMA overlaps the mem scans, and
    mem[1000..1063] += G1*syn_999 + G2*mem_999 with host-precomputed
    geometric tables lands right after the last scan.
  - DMA: x groups alternate the sync/scalar HWDGE queues strictly (each
    HWDGE engine only keeps ~5 DMAs in flight, so <=5 early triggers per
    queue), sizes tuned so both queues carry equal bytes and completion
    order matches program order; the alpha/beta scan broadcast is built
    on-chip by the (idle) VectorE instead of DMAing 0.5 MB.
"""

import numpy as np
import ml_dtypes

B, I, T, O = 256, 182, 2000, 2
NCORES = 8
NB = B // NCORES              # 32 batches per core
M = 2 * NB                    # 64 (batch, o) pairs per time-half
TH = T // 2                   # 1000 time steps per half
NCH = 22                      # full DoubleRow chunks of 8 rows (176 rows)
KTAIL = 3 * NB + 2            # 96 data partitions + 2 bias partitions
LWCOL = 64 + 128 * NCH        # shared-zero-gap stationary layout length
GROUPS = [2, 3, 4, 4, 4, 3, 2]   # chunks per DMA group (sum = NCH)
GQUEUE = [0, 1, 0, 1, 0, 1, 0]   # 0=sync, 1=scalar; strict alternation
TSPLIT = [512, 488]              # PSUM-bank time tiles per half
NCORR = 64                    # carry-correction columns (decay < 1e-7)
NEST = 24                     # syn columns used to estimate mem_999

FP8 = ml_dtypes.float8_e4m3   # TRN FP8_EXP4 (max +-240)

TRACE = False

_cache = {}


def _build_nc():
    import concourse.bacc as bacc
    import concourse.bass as bass
    import concourse.mybir as mybir
    from concourse.tile import TileContext

    f32 = mybir.dt.float32
    fp8 = mybir.dt.float8e4
    DR = mybir.MatmulPerfMode.DoubleRow
    mult, add = mybir.AluOpType.mult, mybir.AluOpType.add

    nc = bacc.Bacc("TRN2", target_bir_lowering=False, debug=False)

    x = nc.dram_tensor("x", [NB, 8 * NCH, T], fp8, kind="ExternalInput")
    x_tail = nc.dram_tensor("x_tail", [KTAIL, 2, T], fp8, kind="ExternalInput")
    lhsT_full = nc.dram_tensor("lhsT_full", [128, 2, LWCOL], fp8, kind="ExternalInput")
    lhsT_tail = nc.dram_tensor("lhsT_tail", [KTAIL, 2, 3 * M], fp8, kind="ExternalInput")
    # packed f32 consts: [:,0,:] G1, [:,1,:] G2 (partitions 64-127);
    # [0:64,2,0:NEST] w24 estimate weights; [:,3,0:2] alpha,beta
    sconst = nc.dram_tensor("sconst", [128, 4, NCORR], f32, kind="ExternalInput")
    y = nc.dram_tensor("y", [M, T], mybir.dt.bfloat16, kind="ExternalOutput")

    with TileContext(nc) as tc:
        with (
            tc.tile_pool(name="consts", bufs=1) as cpool,
            tc.tile_pool(name="xs", bufs=6) as xpool,
            tc.tile_pool(name="xl", bufs=1) as xlpool,
            tc.tile_pool(name="mems", bufs=1) as mpool,
            tc.tile_pool(name="psum", bufs=1, space=bass.MemorySpace.PSUM) as ppool,
        ):
            sco = cpool.tile([128, 4, NCORR], f32)
            nc.scalar.dma_start(out=sco[:], in_=sconst[:])
            lw = cpool.tile([128, 2, LWCOL], fp8)
            nc.scalar.dma_start(out=lw[:], in_=lhsT_full[:])
            lwt = cpool.tile([KTAIL, 2, 3 * M], fp8)
            nc.sync.dma_start(out=lwt[:], in_=lhsT_tail[:])

            # alpha/beta scan broadcasts built on-chip (VectorE is idle
            # during the load phase): abbb[:,k,:] = 0 + sconst[:,3,k]
            ztile = cpool.tile([128, 512], f32)
            nc.vector.memset(ztile[:], 0.0)
            abbb = cpool.tile([128, 2, 512], f32)
            for k in range(2):
                nc.vector.scalar_tensor_tensor(
                    out=abbb[:, k, :],
                    in0=ztile[:],
                    scalar=sco[:, 3, k : k + 1],
                    in1=ztile[:],
                    op0=add,
                    op1=add,
                )

            # one PSUM tile per time-tile: separate tensors so the tile-0
            # scans don't get a false dependency on the PE's tile-1 writes
            pt0 = ppool.tile([128, 512], f32)
            pt1 = ppool.tile([128, 512], f32)
            ptf = ppool.tile([128, 64], f32)
            qs = [nc.sync, nc.scalar]

            def chunk_matmuls(wsel, rhs3, c, tiles):
                """wsel(h) -> [K, 2, 128] stationary window for half h;
                rhs3: [K, 2, T] moving data; one matmul per (tile, half)."""
                for ti in tiles:
                    off = 512 * ti
                    w = TSPLIT[ti]
                    for h in range(2):
                        t0 = TH * h + off
                        nc.tensor.matmul(
                            (pt0, pt1)[ti][:, 0:w],
                            wsel(h),
                            rhs3[:, :, t0 : t0 + w],
                            start=(c == 0 and h == 0),
                            stop=(c == NCH and h == 1),
                            perf_mode=DR,
                        )

            c0 = 0
            for gi, G in enumerate(GROUPS):
                r0 = 8 * c0
                xt = xpool.tile([128, 2 * G, T], fp8, tag="xt")
                src = x[:, r0 : r0 + 8 * G, :].rearrange(
                    "b (i r) t -> b i r t", i=4, r=2 * G
                )
                qs[GQUEUE[gi]].dma_start(out=xt[:], in_=src)
                if gi == 5:
                    # tail chunk data (rows 176..181 regrouped + baked ones
                    # rows for the bias) -- early in the scalar FIFO so the
                    # final accumulation isn't gated on it
                    xe = xlpool.tile([KTAIL, 2, T], fp8)
                    nc.scalar.dma_start(out=xe[:], in_=x_tail[:])
                last = gi == len(GROUPS) - 1

                def wfull(c):
                    return lambda h: lw[:, :, 128 * c + 64 * (1 - h) : 128 * c + 64 * (1 - h) + 128]

                def wtail(h):
                    return lwt[:, :, 64 * (1 - h) : 64 * (1 - h) + 128]

                if gi >= len(GROUPS) - 2:
                    # tiny self-contained filler matmuls: they run during the
                    # DMA wait for this group and keep the PE's HAM clock
                    # gate at K=8/8 (2.4 GHz) for the post-DMA tail burst
                    for _ in range(24):
                        nc.tensor.matmul(
                            ptf[:, 0:64],
                            lw[:, :, 0:128],
                            lw[:, 0:2, 0:64],
                            start=True,
                            stop=True,
                            perf_mode=DR,
                        )
                if not last:
                    for cc in range(G):
                        chunk_matmuls(
                            wfull(c0 + cc),
                            xt[:, 2 * cc : 2 * cc + 2, :],
                            c0 + cc,
                            (0, 1),
                        )
                else:
                    # tile-major for the last group + tail chunk so tile 0's
                    # accumulation closes (and scanning starts) ASAP
                    for ti in range(2):
                        for cc in range(G):
                            chunk_matmuls(
                                wfull(c0 + cc),
                                xt[:, 2 * cc : 2 * cc + 2, :],
                                c0 + cc,
                                (ti,),
                            )
                        chunk_matmuls(wtail, xe[:], NCH, (ti,))
                c0 += G

            syn = mpool.tile([128, TH], f32)
            mem = mpool.tile([128, TH], mybir.dt.bfloat16)
            carry = mpool.tile([128, 2], f32)
            st2 = mpool.tile([128, 2], f32)
            tmp24 = mpool.tile([128, NEST], f32)
            tmp1 = mpool.tile([128, NCORR], f32)

            def scan(out, k, d1, ti):
                off = 512 * ti
                w = TSPLIT[ti]
                nc.vector.tensor_tensor_scan(
                    out[:, off : off + w],
                    abbb[:, k, :w],
                    d1,
                    initial=(0.0 if ti == 0 else out[:, off - 1 : off]),
                    op0=mult,
                    op1=add,
                )

            scan(syn, 0, pt0[:, 0:512], 0)
            scan(mem, 1, syn[:, 0:512], 0)
            # first-half early outputs (everything not behind the carry corr)
            nc.sync.dma_start(out=y[:, :512], in_=mem[0:64, :512])
            nc.scalar.dma_start(
                out=y[:, TH + NCORR : TH + 512], in_=mem[64:128, NCORR:512]
            )
            scan(syn, 0, pt1[:, 0:488], 1)
            # half-boundary carries, computed from syn alone right after the
            # syn scans: copy syn_999 and estimate mem_999 = sum of the last
            # NEST syn columns weighted by beta^(999-s); the partition-shift
            # DMA then overlaps the mem scans.
            nc.vector.scalar_tensor_tensor(
                out=st2[0:64, 0:1],
                in0=syn[0:64, TH - 1 : TH],
                scalar=0.0,
                in1=ztile[0:64, 0:1],
                op0=add,
                op1=add,
            )
            nc.vector.scalar_tensor_tensor(
                out=tmp24[0:64, :],
                in0=syn[0:64, TH - NEST : TH],
                scalar=1.0,
                in1=sco[0:64, 2, 0:NEST],
                op0=mult,
                op1=mult,
                accum_out=st2[0:64, 1:2],
            )
            nc.sync.dma_start(out=carry[64:128, 0:2], in_=st2[0:64, 0:2])
            scan(mem, 1, syn[:, 512:1000], 1)
            # mem[1000+j] += G1[j]*syn_999 + G2[j]*mem_999  (j < NCORR)
            nc.vector.scalar_tensor_tensor(
                out=tmp1[64:128, :],
                in0=sco[64:128, 0, :],
                scalar=carry[64:128, 0:1],
                in1=mem[64:128, :NCORR],
                op0=mult,
                op1=add,
            )
            nc.vector.scalar_tensor_tensor(
                out=mem[64:128, :NCORR],
                in0=sco[64:128, 1, :],
                scalar=carry[64:128, 1:2],
                in1=tmp1[64:128, :],
                op0=mult,
                op1=add,
            )
            # remaining outputs: second tiles of each half, then the NCORR
            # corrected columns last (tiny post-correction tail)
            nc.sync.dma_start(out=y[:, 512:TH], in_=mem[0:64, 512:TH])
            nc.scalar.dma_start(out=y[:, TH + 512 : T], in_=mem[64:128, 512:TH])
            nc.scalar.dma_start(out=y[:, TH : TH + NCORR], in_=mem[64:128, :NCORR])

    nc.compile()
    return nc


def _host_tensors(W, b, alpha, beta):
    """Stationary fp8 weights (shared-zero-gap layout) + packed consts."""
    W = np.asarray(W, np.float32)
    bvec = np.asarray(b, np.float32)
    a_cl = np.clip(np.asarray(alpha, np.float32), 0.0, 1.0)
    bt_cl = np.clip(np.asarray(beta, np.float32), 0.0, 1.0)

    W8 = W.astype(FP8).astype(np.float32)
    bias_fold = (
        bvec.astype(np.float64) + 0.5 * W.astype(np.float64).sum(axis=1)
    ).astype(np.float32)
    bias_hi = bias_fold.astype(FP8).astype(np.float32)
    bias_lo = (bias_fold - bias_hi).astype(FP8).astype(np.float32)

    bidx = np.arange(NB)
    # [z64, W0, z64, W1, ..., z64] with 128 stride between W blocks; the
    # 64-wide zero gaps are shared between half-windows of adjacent chunks
    lhsT = np.zeros((128, 2, LWCOL), np.float32)
    c0 = 0
    for G in GROUPS:
        r0 = 8 * c0
        for cc in range(G):
            c = c0 + cc
            for i in range(4):
                for k in range(2):
                    row = r0 + 2 * G * i + 2 * cc + k
                    for o in range(O):
                        lhsT[4 * bidx + i, k, 128 * c + 64 + 2 * bidx + o] = W8[o, row]
        c0 += G
    assert c0 == NCH

    lhsT_tail = np.zeros((KTAIL, 2, 3 * M), np.float32)
    for i in range(3):
        for k in range(2):
            row = 8 * NCH + 2 * i + k
            for o in range(O):
                lhsT_tail[3 * bidx + i, k, M + 2 * bidx + o] = W8[o, row]
    for o in range(O):
        lhsT_tail[96, 0, M + 2 * bidx + o] = bias_hi[o]
        lhsT_tail[96, 1, M + 2 * bidx + o] = bias_lo[o]

    # packed consts: G1[j] = sum_{s<=j} beta^(j-s)*alpha^(s+1), G2[j] =
    # beta^(j+1) (carry corrections), w24[j] = beta^(NEST-1-j) (mem_999
    # estimate), alpha/beta per partition
    sconst = np.zeros((128, 4, NCORR), np.float32)
    for o in range(O):
        a_, b_ = float(a_cl[o]), float(bt_cl[o])
        g1 = np.empty(NCORR, np.float64)
        acc = 0.0
        apow = 1.0
        for j in range(NCORR):
            apow *= a_
            acc = b_ * acc + apow
            g1[j] = acc
        sconst[64 + o :: 2, 0, :][:] = g1.astype(np.float32)[None, :][
            :, : NCORR
        ]
        sconst[64 + o :: 2, 1, :] = (
            b_ ** np.arange(1, NCORR + 1, dtype=np.float64)
        ).astype(np.float32)
        sconst[o:64:2, 2, 0:NEST] = (
            b_ ** np.arange(NEST - 1, -1, -1, dtype=np.float64)
        ).astype(np.float32)
        sconst[o::2, 3, 0] = a_
        sconst[o::2, 3, 1] = b_

    return lhsT.astype(FP8), lhsT_tail.astype(FP8), np.ascontiguousarray(sconst)


def kernel(inputs, W, b, alpha, beta):
    from concourse.bass_utils import run_bass_kernel_spmd

    if "nc" not in _cache:
        _cache["nc"] = _build_nc()
    nc = _cache["nc"]

    lhsT_full, lhsT_tail, sconst = _host_tensors(W, b, alpha, beta)

    x_c = (np.asarray(inputs, np.float32) - np.float32(0.5)).astype(FP8)  # [B, I, T]

    in_maps = []
    for c in range(NCORES):
        xc = x_c[c * NB : (c + 1) * NB]
        # tail rows 176..182 regrouped to [96, 2, T] + two ones rows for bias
        xt = np.empty((KTAIL, 2, T), FP8)
        xt[:96] = xc[:, 176:182, :].reshape(NB * 3, 2, T)
        xt[96:] = np.float32(1.0)
        in_maps.append(
            {
                "x": np.ascontiguousarray(xc[:, : 8 * NCH]),
                "x_tail": xt,
                "lhsT_full": lhsT_full,
                "lhsT_tail": lhsT_tail,
                "sconst": sconst,
            }
        )

    res = run_bass_kernel_spmd(nc, in_maps, core_ids=list(range(NCORES)), trace=TRACE)
    kernel.last_exec_time_ns = res.exec_time_ns
    kernel.last_result = res
    out = np.empty((B, O, T), np.float32)
    for c in range(NCORES):
        out[c * NB : (c + 1) * NB] = res.results[c]["y"].astype(np.float32).reshape(
            NB, O, T
        )
    return np.ascontiguousarray(out.transpose(0, 2, 1))


kernel.last_exec_time_ns = None
kernel.last_result = None


# revision 48
# speedup vs baseline: 1.0368x; 1.0239x over previous
"""Trainium2 Bass kernel for the DecoderSVM SNN decoder.

reference computation:
    curr[t,b,o] = einsum('bit,oi->tbo', inputs, W) + b         (I=182 -> O=2)
    syn_t = clip(alpha,0,1)*syn_{t-1} + curr_t                 (scan over T)
    mem_t = clip(beta,0,1)*mem_{t-1} + syn_t
    out = mem_rec transposed to [B, T, O]

Strategy (8 NeuronCores, batch-sharded 32 per core), fp8 DoubleRow edition:
  - Inputs are centered (x - 0.5) and cast to fp8_e4m3; the exact mean
    term 0.5*sum_i W[o,i] + b[o] is folded into a bias constant host-side
    (in f32), so fp8's coarse mantissa only touches the zero-mean part.
    Measured end-to-end rel err ~1.04e-2 vs the 2e-2 gate.
  - fp8 halves HBM traffic (11.65 MB/core) and DoubleRow matmul
    (perf_mode, 2 fp8 MACs/partition/cycle, K-tiles of 2) nearly halves PE
    time: 8 input rows per chunk -> 22 full chunks + 1 tail chunk.
  - Block-diagonal stationary lhsT [128, 2, 64-wide W block]: K = 32
    batches x 4 partition-rows (x 2 k-tiles), 64 (batch, o) outputs.
  - The bias constant rides in the tail chunk as two extra K partitions
    (96: hi, 97: lo in fp8) against host-baked ones rows.
  - Time axis split in half across PSUM partitions: partitions 0-63 hold
    t in [0,1000), partitions 64-127 hold t in [1000,2000).  The ISA
    rejects DoubleRow matmuls at a column tile offset, so every matmul is
    full-PE [128, w] with a zero-padded 128-wide weight window: the
    stationary buffer interleaves 64-wide W blocks with 64-wide zero gaps
    ([z64, W0, z64, W1, ...], stride 128), and half h of chunk c slices
    window [128c + 64(1-h) : +128].  The wrong half accumulates zeros.
  - Both halves scan in parallel in single tensor_tensor_scan calls (the
    scan is the serial tail; this halves it).  The half-boundary carry is
    fixed up at the end: syn_999 is copied and mem_999 estimated from the
    last 24 syn columns (beta^24 < 1e-6) right after the syn scans, so
    one tiny partition-shift DMA overlaps the mem scans, and
    mem[1000..1063] += G1*syn_999 + G2*mem_999 with host-precomputed
    geometric tables lands right after the last scan.
  - DMA: x groups alternate the sync/scalar HWDGE queues strictly (each
    HWDGE engine only keeps ~5 DMAs in flight, so <=5 early triggers per
    queue), sizes tuned so both queues carry equal bytes and completion
    order matches program order; the alpha/beta scan broadcast is built
    on-chip by the (idle) VectorE instead of DMAing 0.5 MB.
"""

import numpy as np
import ml_dtypes

B, I, T, O = 256, 182, 2000, 2
NCORES = 8
NB = B // NCORES              # 32 batches per core
M = 2 * NB                    # 64 (batch, o) pairs per time-half
TH = T // 2                   # 1000 time steps per half
NCH = 22                      # full DoubleRow chunks of 8 rows (176 rows)
KTAIL = 3 * NB + 2            # 96 data partitions + 2 bias partitions
LWCOL = 64 + 128 * NCH        # shared-zero-gap stationary layout length
GROUPS = [2, 3, 4, 4, 4, 3, 2]   # chunks per DMA group (sum = NCH)
GQUEUE = [0, 1, 0, 1, 0, 1, 0]   # 0=sync, 1=scalar; strict alternation
TSPLIT = [512, 488]              # PSUM-bank time tiles per half
NCORR = 64                    # carry-correction columns (decay < 1e-7)
NEST = 24                     # syn columns used to estimate mem_999

FP8 = ml_dtypes.float8_e4m3   # TRN FP8_EXP4 (max +-240)

TRACE = False

_cache = {}


def _build_nc():
    import concourse.bacc as bacc
    import concourse.bass as bass
    import concourse.mybir as mybir
    from concourse.tile import TileContext

    f32 = mybir.dt.float32
    fp8 = mybir.dt.float8e4
    DR = mybir.MatmulPerfMode.DoubleRow
    mult, add = mybir.AluOpType.mult, mybir.AluOpType.add

    nc = bacc.Bacc("TRN2", target_bir_lowering=False, debug=False)

    x = nc.dram_tensor("x", [NB, 8 * NCH, T], fp8, kind="ExternalInput")
    x_tail = nc.dram_tensor("x_tail", [KTAIL, 2, T], fp8, kind="ExternalInput")
    lhsT_full = nc.dram_tensor("lhsT_full", [128, 2, LWCOL], fp8, kind="ExternalInput")
    lhsT_tail = nc.dram_tensor("lhsT_tail", [KTAIL, 2, 3 * M], fp8, kind="ExternalInput")
    # packed f32 consts: [:,0,:] G1, [:,1,:] G2 (partitions 64-127);
    # [0:64,2,0:NEST] w24 estimate weights; [:,3,0:2] alpha,beta
    sconst = nc.dram_tensor("sconst", [128, 4, NCORR], f32, kind="ExternalInput")
    y = nc.dram_tensor("y", [M, T], mybir.dt.bfloat16, kind="ExternalOutput")

    with TileContext(nc) as tc:
        with (
            tc.tile_pool(name="consts", bufs=1) as cpool,
            tc.tile_pool(name="xs", bufs=6) as xpool,
            tc.tile_pool(name="xl", bufs=1) as xlpool,
            tc.tile_pool(name="mems", bufs=1) as mpool,
            tc.tile_pool(name="psum", bufs=1, space=bass.MemorySpace.PSUM) as ppool,
        ):
            sco = cpool.tile([128, 4, NCORR], f32)
            nc.scalar.dma_start(out=sco[:], in_=sconst[:])
            lw = cpool.tile([128, 2, LWCOL], fp8)
            nc.scalar.dma_start(out=lw[:], in_=lhsT_full[:])
            lwt = cpool.tile([KTAIL, 2, 3 * M], fp8)
            nc.sync.dma_start(out=lwt[:], in_=lhsT_tail[:])

            # alpha/beta scan broadcasts built on-chip (VectorE is idle
            # during the load phase): abbb[:,k,:] = 0 + sconst[:,3,k]
            ztile = cpool.tile([128, 512], f32)
            nc.vector.memset(ztile[:], 0.0)
            aal = cpool.tile([128, 512], f32)
            bbt = cpool.tile([128, 512], mybir.dt.bfloat16)
            for k, dst in ((0, aal), (1, bbt)):
                nc.vector.scalar_tensor_tensor(
                    out=dst[:],
                    in0=ztile[:],
                    scalar=sco[:, 3, k : k + 1],
                    in1=ztile[:],
                    op0=add,
                    op1=add,
                )

            # one PSUM tile per time-tile: separate tensors so the tile-0
            # scans don't get a false dependency on the PE's tile-1 writes
            pt0 = ppool.tile([128, 512], f32)
            pt1 = ppool.tile([128, 512], f32)
            ptf = ppool.tile([128, 64], f32)
            qs = [nc.sync, nc.scalar]

            def chunk_matmuls(wsel, rhs3, c, tiles):
                """wsel(h) -> [K, 2, 128] stationary window for half h;
                rhs3: [K, 2, T] moving data; one matmul per (tile, half)."""
                for ti in tiles:
                    off = 512 * ti
                    w = TSPLIT[ti]
                    for h in range(2):
                        t0 = TH * h + off
                        nc.tensor.matmul(
                            (pt0, pt1)[ti][:, 0:w],
                            wsel(h),
                            rhs3[:, :, t0 : t0 + w],
                            start=(c == 0 and h == 0),
                            stop=(c == NCH and h == 1),
                            perf_mode=DR,
                        )

            c0 = 0
            for gi, G in enumerate(GROUPS):
                r0 = 8 * c0
                xt = xpool.tile([128, 2 * G, T], fp8, tag="xt")
                src = x[:, r0 : r0 + 8 * G, :].rearrange(
                    "b (i r) t -> b i r t", i=4, r=2 * G
                )
                qs[GQUEUE[gi]].dma_start(out=xt[:], in_=src)
                if gi == 5:
                    # tail chunk data (rows 176..181 regrouped + baked ones
                    # rows for the bias) -- early in the scalar FIFO so the
                    # final accumulation isn't gated on it
                    xe = xlpool.tile([KTAIL, 2, T], fp8)
                    nc.scalar.dma_start(out=xe[:], in_=x_tail[:])
                last = gi == len(GROUPS) - 1

                def wfull(c):
                    return lambda h: lw[:, :, 128 * c + 64 * (1 - h) : 128 * c + 64 * (1 - h) + 128]

                def wtail(h):
                    return lwt[:, :, 64 * (1 - h) : 64 * (1 - h) + 128]

                if gi >= len(GROUPS) - 2:
                    # tiny self-contained filler matmuls: they run during the
                    # DMA wait for this group and keep the PE's HAM clock
                    # gate at K=8/8 (2.4 GHz) for the post-DMA tail burst
                    for _ in range(24):
                        nc.tensor.matmul(
                            ptf[:, 0:64],
                            lw[:, :, 0:128],
                            lw[:, 0:2, 0:64],
                            start=True,
                            stop=True,
                            perf_mode=DR,
                        )
                if not last:
                    for cc in range(G):
                        chunk_matmuls(
                            wfull(c0 + cc),
                            xt[:, 2 * cc : 2 * cc + 2, :],
                            c0 + cc,
                            (0, 1),
                        )
                else:
                    # tile-major for the last group + tail chunk so tile 0's
                    # accumulation closes (and scanning starts) ASAP
                    for ti in range(2):
                        for cc in range(G):
                            chunk_matmuls(
                                wfull(c0 + cc),
                                xt[:, 2 * cc : 2 * cc + 2, :],
                                c0 + cc,
                                (ti,),
                            )
                        chunk_matmuls(wtail, xe[:], NCH, (ti,))
                c0 += G

            syn = mpool.tile([128, TH], mybir.dt.bfloat16)
            mem = mpool.tile([128, TH], mybir.dt.bfloat16)
            carry = mpool.tile([128, 2], f32)
            st2 = mpool.tile([128, 2], f32)
            tmp24 = mpool.tile([128, NEST], f32)
            tmp1 = mpool.tile([128, NCORR], f32)

            def scan(out, k, d1, ti):
                off = 512 * ti
                w = TSPLIT[ti]
                nc.vector.tensor_tensor_scan(
                    out[:, off : off + w],
                    (aal, bbt)[k][:, :w],
                    d1,
                    initial=(0.0 if ti == 0 else out[:, off - 1 : off]),
                    op0=mult,
                    op1=add,
                )

            scan(syn, 0, pt0[:, 0:512], 0)
            scan(mem, 1, syn[:, 0:512], 0)
            # first-half early outputs (everything not behind the carry corr)
            nc.sync.dma_start(out=y[:, :512], in_=mem[0:64, :512])
            nc.scalar.dma_start(
                out=y[:, TH + NCORR : TH + 512], in_=mem[64:128, NCORR:512]
            )
            scan(syn, 0, pt1[:, 0:488], 1)
            # half-boundary carries, computed from syn alone right after the
            # syn scans: copy syn_999 and estimate mem_999 = sum of the last
            # NEST syn columns weighted by beta^(999-s); the partition-shift
            # DMA then overlaps the mem scans.
            nc.vector.scalar_tensor_tensor(
                out=st2[0:64, 0:1],
                in0=syn[0:64, TH - 1 : TH],
                scalar=0.0,
                in1=ztile[0:64, 0:1],
                op0=add,
                op1=add,
            )
            nc.vector.scalar_tensor_tensor(
                out=tmp24[0:64, :],
                in0=syn[0:64, TH - NEST : TH],
                scalar=1.0,
                in1=sco[0:64, 2, 0:NEST],
                op0=mult,
                op1=mult,
                accum_out=st2[0:64, 1:2],
            )
            nc.sync.dma_start(out=carry[64:128, 0:2], in_=st2[0:64, 0:2])
            scan(mem, 1, syn[:, 512:1000], 1)
            # mem[1000+j] += G1[j]*syn_999 + G2[j]*mem_999  (j < NCORR)
            nc.vector.scalar_tensor_tensor(
                out=tmp1[64:128, :],
                in0=sco[64:128, 0, :],
                scalar=carry[64:128, 0:1],
                in1=mem[64:128, :NCORR],
                op0=mult,
                op1=add,
            )
            nc.vector.scalar_tensor_tensor(
                out=mem[64:128, :NCORR],
                in0=sco[64:128, 1, :],
                scalar=carry[64:128, 1:2],
                in1=tmp1[64:128, :],
                op0=mult,
                op1=add,
            )
            # remaining outputs: second tiles of each half, then the NCORR
            # corrected columns last (tiny post-correction tail)
            nc.sync.dma_start(out=y[:, 512:TH], in_=mem[0:64, 512:TH])
            nc.scalar.dma_start(out=y[:, TH + 512 : T], in_=mem[64:128, 512:TH])
            nc.scalar.dma_start(out=y[:, TH : TH + NCORR], in_=mem[64:128, :NCORR])

    nc.compile()
    return nc


def _host_tensors(W, b, alpha, beta):
    """Stationary fp8 weights (shared-zero-gap layout) + packed consts."""
    W = np.asarray(W, np.float32)
    bvec = np.asarray(b, np.float32)
    a_cl = np.clip(np.asarray(alpha, np.float32), 0.0, 1.0)
    bt_cl = np.clip(np.asarray(beta, np.float32), 0.0, 1.0)

    W8 = W.astype(FP8).astype(np.float32)
    bias_fold = (
        bvec.astype(np.float64) + 0.5 * W.astype(np.float64).sum(axis=1)
    ).astype(np.float32)
    bias_hi = bias_fold.astype(FP8).astype(np.float32)
    bias_lo = (bias_fold - bias_hi).astype(FP8).astype(np.float32)

    bidx = np.arange(NB)
    # [z64, W0, z64, W1, ..., z64] with 128 stride between W blocks; the
    # 64-wide zero gaps are shared between half-windows of adjacent chunks
    lhsT = np.zeros((128, 2, LWCOL), np.float32)
    c0 = 0
    for G in GROUPS:
        r0 = 8 * c0
        for cc in range(G):
            c = c0 + cc
            for i in range(4):
                for k in range(2):
                    row = r0 + 2 * G * i + 2 * cc + k
                    for o in range(O):
                        lhsT[4 * bidx + i, k, 128 * c + 64 + 2 * bidx + o] = W8[o, row]
        c0 += G
    assert c0 == NCH

    lhsT_tail = np.zeros((KTAIL, 2, 3 * M), np.float32)
    for i in range(3):
        for k in range(2):
            row = 8 * NCH + 2 * i + k
            for o in range(O):
                lhsT_tail[3 * bidx + i, k, M + 2 * bidx + o] = W8[o, row]
    for o in range(O):
        lhsT_tail[96, 0, M + 2 * bidx + o] = bias_hi[o]
        lhsT_tail[96, 1, M + 2 * bidx + o] = bias_lo[o]

    # packed consts: G1[j] = sum_{s<=j} beta^(j-s)*alpha^(s+1), G2[j] =
    # beta^(j+1) (carry corrections), w24[j] = beta^(NEST-1-j) (mem_999
    # estimate), alpha/beta per partition
    sconst = np.zeros((128, 4, NCORR), np.float32)
    for o in range(O):
        a_, b_ = float(a_cl[o]), float(bt_cl[o])
        g1 = np.empty(NCORR, np.float64)
        acc = 0.0
        apow = 1.0
        for j in range(NCORR):
            apow *= a_
            acc = b_ * acc + apow
            g1[j] = acc
        sconst[64 + o :: 2, 0, :][:] = g1.astype(np.float32)[None, :][
            :, : NCORR
        ]
        sconst[64 + o :: 2, 1, :] = (
            b_ ** np.arange(1, NCORR + 1, dtype=np.float64)
        ).astype(np.float32)
        sconst[o:64:2, 2, 0:NEST] = (
            b_ ** np.arange(NEST - 1, -1, -1, dtype=np.float64)
        ).astype(np.float32)
        sconst[o::2, 3, 0] = a_
        sconst[o::2, 3, 1] = b_

    return lhsT.astype(FP8), lhsT_tail.astype(FP8), np.ascontiguousarray(sconst)


def kernel(inputs, W, b, alpha, beta):
    from concourse.bass_utils import run_bass_kernel_spmd

    if "nc" not in _cache:
        _cache["nc"] = _build_nc()
    nc = _cache["nc"]

    lhsT_full, lhsT_tail, sconst = _host_tensors(W, b, alpha, beta)

    x_c = (np.asarray(inputs, np.float32) - np.float32(0.5)).astype(FP8)  # [B, I, T]

    in_maps = []
    for c in range(NCORES):
        xc = x_c[c * NB : (c + 1) * NB]
        # tail rows 176..182 regrouped to [96, 2, T] + two ones rows for bias
        xt = np.empty((KTAIL, 2, T), FP8)
        xt[:96] = xc[:, 176:182, :].reshape(NB * 3, 2, T)
        xt[96:] = np.float32(1.0)
        in_maps.append(
            {
                "x": np.ascontiguousarray(xc[:, : 8 * NCH]),
                "x_tail": xt,
                "lhsT_full": lhsT_full,
                "lhsT_tail": lhsT_tail,
                "sconst": sconst,
            }
        )

    res = run_bass_kernel_spmd(nc, in_maps, core_ids=list(range(NCORES)), trace=TRACE)
    kernel.last_exec_time_ns = res.exec_time_ns
    kernel.last_result = res
    out = np.empty((B, O, T), np.float32)
    for c in range(NCORES):
        out[c * NB : (c + 1) * NB] = res.results[c]["y"].astype(np.float32).reshape(
            NB, O, T
        )
    return np.ascontiguousarray(out.transpose(0, 2, 1))


kernel.last_exec_time_ns = None
kernel.last_result = None


# revision 49
# speedup vs baseline: 1.0835x; 1.0451x over previous
"""Trainium2 Bass kernel for the DecoderSVM SNN decoder.

reference computation:
    curr[t,b,o] = einsum('bit,oi->tbo', inputs, W) + b         (I=182 -> O=2)
    syn_t = clip(alpha,0,1)*syn_{t-1} + curr_t                 (scan over T)
    mem_t = clip(beta,0,1)*mem_{t-1} + syn_t
    out = mem_rec transposed to [B, T, O]

Strategy (8 NeuronCores, batch-sharded 32 per core), fp8 DoubleRow edition:
  - Inputs are centered (x - 0.5) and cast to fp8_e4m3; the exact mean
    term 0.5*sum_i W[o,i] + b[o] is folded into a bias constant host-side
    (in f32), so fp8's coarse mantissa only touches the zero-mean part.
    Measured end-to-end rel err ~1.04e-2 vs the 2e-2 gate.
  - fp8 halves HBM traffic (11.65 MB/core) and DoubleRow matmul
    (perf_mode, 2 fp8 MACs/partition/cycle, K-tiles of 2) nearly halves PE
    time: 8 input rows per chunk -> 22 full chunks + 1 tail chunk.
  - Block-diagonal stationary lhsT [128, 2, 64-wide W block]: K = 32
    batches x 4 partition-rows (x 2 k-tiles), 64 (batch, o) outputs.
  - The bias constant rides in the tail chunk as two extra K partitions
    (96: hi, 97: lo in fp8) against host-baked ones rows.
  - Time axis split in half across PSUM partitions: partitions 0-63 hold
    t in [0,1000), partitions 64-127 hold t in [1000,2000).  The ISA
    rejects DoubleRow matmuls at a column tile offset, so every matmul is
    full-PE [128, w] with a zero-padded 128-wide weight window: the
    stationary buffer interleaves 64-wide W blocks with 64-wide zero gaps
    ([z64, W0, z64, W1, ...], stride 128), and half h of chunk c slices
    window [128c + 64(1-h) : +128].  The wrong half accumulates zeros.
  - Both halves scan in parallel in single tensor_tensor_scan calls (the
    scan is the serial tail; this halves it).  The half-boundary carry is
    fixed up at the end: syn_999 is copied and mem_999 estimated from the
    last 24 syn columns (beta^24 < 1e-6) right after the syn scans, so
    one tiny partition-shift DMA overlaps the mem scans, and
    mem[1000..1063] += G1*syn_999 + G2*mem_999 with host-precomputed
    geometric tables lands right after the last scan.
  - DMA: x groups alternate the sync/scalar HWDGE queues strictly (each
    HWDGE engine only keeps ~5 DMAs in flight, so <=5 early triggers per
    queue), sizes tuned so both queues carry equal bytes and completion
    order matches program order; the alpha/beta scan broadcast is built
    on-chip by the (idle) VectorE instead of DMAing 0.5 MB.
"""

import numpy as np
import ml_dtypes

B, I, T, O = 256, 182, 2000, 2
NCORES = 8
NB = B // NCORES              # 32 batches per core
M = 2 * NB                    # 64 (batch, o) pairs per time-half
TH = T // 2                   # 1000 time steps per half
NCH = 22                      # full DoubleRow chunks of 8 rows (176 rows)
KTAIL = 3 * NB + 2            # 96 data partitions + 2 bias partitions
LWCOL = 64 + 128 * NCH        # shared-zero-gap stationary layout length
GROUPS = [2, 3, 4, 4, 4, 3, 2]   # chunks per DMA group (sum = NCH)
GQUEUE = [0, 1, 0, 1, 0, 1, 0]   # 0=sync, 1=scalar; strict alternation
TSPLIT = [512, 488]              # PSUM-bank time tiles per half
NCORR = 64                    # carry-correction columns (decay < 1e-7)
NEST = 24                     # syn columns used to estimate mem_999

FP8 = ml_dtypes.float8_e4m3   # TRN FP8_EXP4 (max +-240)

TRACE = False

_cache = {}


def _build_nc():
    import concourse.bacc as bacc
    import concourse.bass as bass
    import concourse.mybir as mybir
    from concourse.tile import TileContext

    f32 = mybir.dt.float32
    fp8 = mybir.dt.float8e4
    DR = mybir.MatmulPerfMode.DoubleRow
    mult, add = mybir.AluOpType.mult, mybir.AluOpType.add

    nc = bacc.Bacc("TRN2", target_bir_lowering=False, debug=False)

    x = nc.dram_tensor("x", [NB, 8 * NCH, T], fp8, kind="ExternalInput")
    x_tail = nc.dram_tensor("x_tail", [KTAIL, 2, T], fp8, kind="ExternalInput")
    lhsT_full = nc.dram_tensor("lhsT_full", [128, 2, LWCOL], fp8, kind="ExternalInput")
    lhsT_tail = nc.dram_tensor("lhsT_tail", [KTAIL, 2, 3 * M], fp8, kind="ExternalInput")
    # packed f32 consts: [:,0,:] G1, [:,1,:] G2 (partitions 64-127);
    # [0:64,2,0:NEST] w24 estimate weights; [:,3,0:2] alpha,beta
    sconst = nc.dram_tensor("sconst", [128, 4, NCORR], f32, kind="ExternalInput")
    y = nc.dram_tensor("y", [M, T], mybir.dt.bfloat16, kind="ExternalOutput")

    with TileContext(nc) as tc:
        with (
            tc.tile_pool(name="consts", bufs=1) as cpool,
            tc.tile_pool(name="xs", bufs=6) as xpool,
            tc.tile_pool(name="xl", bufs=1) as xlpool,
            tc.tile_pool(name="mems", bufs=1) as mpool,
            tc.tile_pool(name="psum", bufs=1, space=bass.MemorySpace.PSUM) as ppool,
        ):
            sco = cpool.tile([128, 4, NCORR], f32)
            nc.scalar.dma_start(out=sco[:], in_=sconst[:])
            lw = cpool.tile([128, 2, LWCOL], fp8)
            nc.scalar.dma_start(out=lw[:], in_=lhsT_full[:])
            lwt = cpool.tile([KTAIL, 2, 3 * M], fp8)
            nc.sync.dma_start(out=lwt[:], in_=lhsT_tail[:])

            # alpha/beta scan broadcasts built on-chip (VectorE is idle
            # during the load phase): abbb[:,k,:] = 0 + sconst[:,3,k]
            ztile = cpool.tile([128, 512], f32)
            nc.vector.memset(ztile[:], 0.0)
            abbb = cpool.tile([128, 2, 512], f32)
            for k in range(2):
                nc.vector.scalar_tensor_tensor(
                    out=abbb[:, k, :],
                    in0=ztile[:],
                    scalar=sco[:, 3, k : k + 1],
                    in1=ztile[:],
                    op0=add,
                    op1=add,
                )

            # one PSUM tile per time-tile: separate tensors so the tile-0
            # scans don't get a false dependency on the PE's tile-1 writes
            pt0 = ppool.tile([128, 512], f32)
            pt1 = ppool.tile([128, 512], f32)
            ptf = ppool.tile([128, 64], f32)
            qs = [nc.sync, nc.scalar]

            def chunk_matmuls(wsel, rhs3, c, tiles):
                """wsel(h) -> [K, 2, 128] stationary window for half h;
                rhs3: [K, 2, T] moving data; one matmul per (tile, half)."""
                for ti in tiles:
                    off = 512 * ti
                    w = TSPLIT[ti]
                    for h in range(2):
                        t0 = TH * h + off
                        nc.tensor.matmul(
                            (pt0, pt1)[ti][:, 0:w],
                            wsel(h),
                            rhs3[:, :, t0 : t0 + w],
                            start=(c == 0 and h == 0),
                            stop=(c == NCH and h == 1),
                            perf_mode=DR,
                        )

            c0 = 0
            for gi, G in enumerate(GROUPS):
                r0 = 8 * c0
                xt = xpool.tile([128, 2 * G, T], fp8, tag="xt")
                src = x[:, r0 : r0 + 8 * G, :].rearrange(
                    "b (i r) t -> b i r t", i=4, r=2 * G
                )
                qs[GQUEUE[gi]].dma_start(out=xt[:], in_=src)
                if gi == 5:
                    # tail chunk data (rows 176..181 regrouped + baked ones
                    # rows for the bias) -- early in the scalar FIFO so the
                    # final accumulation isn't gated on it
                    xe = xlpool.tile([KTAIL, 2, T], fp8)
                    nc.scalar.dma_start(out=xe[:], in_=x_tail[:])
                last = gi == len(GROUPS) - 1

                def wfull(c):
                    return lambda h: lw[:, :, 128 * c + 64 * (1 - h) : 128 * c + 64 * (1 - h) + 128]

                def wtail(h):
                    return lwt[:, :, 64 * (1 - h) : 64 * (1 - h) + 128]

                if gi >= len(GROUPS) - 2:
                    # tiny self-contained filler matmuls: they run during the
                    # DMA wait for this group and keep the PE's HAM clock
                    # gate at K=8/8 (2.4 GHz) for the post-DMA tail burst
                    for _ in range(24):
                        nc.tensor.matmul(
                            ptf[:, 0:64],
                            lw[:, :, 0:128],
                            lw[:, 0:2, 0:64],
                            start=True,
                            stop=True,
                            perf_mode=DR,
                        )
                if not last:
                    for cc in range(G):
                        chunk_matmuls(
                            wfull(c0 + cc),
                            xt[:, 2 * cc : 2 * cc + 2, :],
                            c0 + cc,
                            (0, 1),
                        )
                else:
                    # tile-major for the last group + tail chunk so tile 0's
                    # accumulation closes (and scanning starts) ASAP
                    for ti in range(2):
                        for cc in range(G):
                            chunk_matmuls(
                                wfull(c0 + cc),
                                xt[:, 2 * cc : 2 * cc + 2, :],
                                c0 + cc,
                                (ti,),
                            )
                        chunk_matmuls(wtail, xe[:], NCH, (ti,))
                c0 += G

            syn = mpool.tile([128, TH], f32)
            mem = mpool.tile([128, TH], mybir.dt.bfloat16)
            carry = mpool.tile([128, 2], f32)
            st2 = mpool.tile([128, 2], f32)
            tmp24 = mpool.tile([128, NEST], f32)
            tmp1 = mpool.tile([128, NCORR], f32)

            def scan(out, k, d1, ti):
                off = 512 * ti
                w = TSPLIT[ti]
                nc.vector.tensor_tensor_scan(
                    out[:, off : off + w],
                    abbb[:, k, :w],
                    d1,
                    initial=(0.0 if ti == 0 else out[:, off - 1 : off]),
                    op0=mult,
                    op1=add,
                )

            scan(syn, 0, pt0[:, 0:512], 0)
            scan(mem, 1, syn[:, 0:512], 0)
            # first-half early outputs (everything not behind the carry corr)
            nc.sync.dma_start(out=y[:, :512], in_=mem[0:64, :512])
            nc.scalar.dma_start(
                out=y[:, TH + NCORR : TH + 512], in_=mem[64:128, NCORR:512]
            )
            scan(syn, 0, pt1[:, 0:488], 1)
            # half-boundary carries, computed from syn alone right after the
            # syn scans: copy syn_999 and estimate mem_999 = sum of the last
            # NEST syn columns weighted by beta^(999-s); the partition-shift
            # DMA then overlaps the mem scans.
            nc.vector.scalar_tensor_tensor(
                out=st2[0:64, 0:1],
                in0=syn[0:64, TH - 1 : TH],
                scalar=0.0,
                in1=ztile[0:64, 0:1],
                op0=add,
                op1=add,
            )
            nc.vector.scalar_tensor_tensor(
                out=tmp24[0:64, :],
                in0=syn[0:64, TH - NEST : TH],
                scalar=1.0,
                in1=sco[0:64, 2, 0:NEST],
                op0=mult,
                op1=mult,
                accum_out=st2[0:64, 1:2],
            )
            nc.sync.dma_start(out=carry[64:128, 0:2], in_=st2[0:64, 0:2])
            scan(mem, 1, syn[:, 512:1000], 1)
            # mem[1000+j] += G1[j]*syn_999 + G2[j]*mem_999  (j < NCORR)
            nc.vector.scalar_tensor_tensor(
                out=tmp1[64:128, :],
                in0=sco[64:128, 0, :],
                scalar=carry[64:128, 0:1],
                in1=mem[64:128, :NCORR],
                op0=mult,
                op1=add,
            )
            nc.vector.scalar_tensor_tensor(
                out=mem[64:128, :NCORR],
                in0=sco[64:128, 1, :],
                scalar=carry[64:128, 1:2],
                in1=tmp1[64:128, :],
                op0=mult,
                op1=add,
            )
            # remaining outputs: second tiles of each half, then the NCORR
            # corrected columns last (tiny post-correction tail)
            nc.sync.dma_start(out=y[:, 512:TH], in_=mem[0:64, 512:TH])
            nc.scalar.dma_start(out=y[:, TH + 512 : T], in_=mem[64:128, 512:TH])
            nc.scalar.dma_start(out=y[:, TH : TH + NCORR], in_=mem[64:128, :NCORR])

    nc.compile()
    return nc


def _host_tensors(W, b, alpha, beta):
    """Stationary fp8 weights (shared-zero-gap layout) + packed consts."""
    W = np.asarray(W, np.float32)
    bvec = np.asarray(b, np.float32)
    a_cl = np.clip(np.asarray(alpha, np.float32), 0.0, 1.0)
    bt_cl = np.clip(np.asarray(beta, np.float32), 0.0, 1.0)

    W8 = W.astype(FP8).astype(np.float32)
    bias_fold = (
        bvec.astype(np.float64) + 0.5 * W.astype(np.float64).sum(axis=1)
    ).astype(np.float32)
    bias_hi = bias_fold.astype(FP8).astype(np.float32)
    bias_lo = (bias_fold - bias_hi).astype(FP8).astype(np.float32)

    bidx = np.arange(NB)
    # [z64, W0, z64, W1, ..., z64] with 128 stride between W blocks; the
    # 64-wide zero gaps are shared between half-windows of adjacent chunks
    lhsT = np.zeros((128, 2, LWCOL), np.float32)
    c0 = 0
    for G in GROUPS:
        r0 = 8 * c0
        for cc in range(G):
            c = c0 + cc
            for i in range(4):
                for k in range(2):
                    row = r0 + 2 * G * i + 2 * cc + k
                    for o in range(O):
                        lhsT[4 * bidx + i, k, 128 * c + 64 + 2 * bidx + o] = W8[o, row]
        c0 += G
    assert c0 == NCH

    lhsT_tail = np.zeros((KTAIL, 2, 3 * M), np.float32)
    for i in range(3):
        for k in range(2):
            row = 8 * NCH + 2 * i + k
            for o in range(O):
                lhsT_tail[3 * bidx + i, k, M + 2 * bidx + o] = W8[o, row]
    for o in range(O):
        lhsT_tail[96, 0, M + 2 * bidx + o] = bias_hi[o]
        lhsT_tail[96, 1, M + 2 * bidx + o] = bias_lo[o]

    # packed consts: G1[j] = sum_{s<=j} beta^(j-s)*alpha^(s+1), G2[j] =
    # beta^(j+1) (carry corrections), w24[j] = beta^(NEST-1-j) (mem_999
    # estimate), alpha/beta per partition
    sconst = np.zeros((128, 4, NCORR), np.float32)
    for o in range(O):
        a_, b_ = float(a_cl[o]), float(bt_cl[o])
        g1 = np.empty(NCORR, np.float64)
        acc = 0.0
        apow = 1.0
        for j in range(NCORR):
            apow *= a_
            acc = b_ * acc + apow
            g1[j] = acc
        sconst[64 + o :: 2, 0, :][:] = g1.astype(np.float32)[None, :][
            :, : NCORR
        ]
        sconst[64 + o :: 2, 1, :] = (
            b_ ** np.arange(1, NCORR + 1, dtype=np.float64)
        ).astype(np.float32)
        sconst[o:64:2, 2, 0:NEST] = (
            b_ ** np.arange(NEST - 1, -1, -1, dtype=np.float64)
        ).astype(np.float32)
        sconst[o::2, 3, 0] = a_
        sconst[o::2, 3, 1] = b_

    return lhsT.astype(FP8), lhsT_tail.astype(FP8), np.ascontiguousarray(sconst)


def kernel(inputs, W, b, alpha, beta):
    from concourse.bass_utils import run_bass_kernel_spmd

    if "nc" not in _cache:
        _cache["nc"] = _build_nc()
    nc = _cache["nc"]

    lhsT_full, lhsT_tail, sconst = _host_tensors(W, b, alpha, beta)

    x_c = (np.asarray(inputs, np.float32) - np.float32(0.5)).astype(FP8)  # [B, I, T]

    in_maps = []
    for c in range(NCORES):
        xc = x_c[c * NB : (c + 1) * NB]
        # tail rows 176..182 regrouped to [96, 2, T] + two ones rows for bias
        xt = np.empty((KTAIL, 2, T), FP8)
        xt[:96] = xc[:, 176:182, :].reshape(NB * 3, 2, T)
        xt[96:] = np.float32(1.0)
        in_maps.append(
            {
                "x": np.ascontiguousarray(xc[:, : 8 * NCH]),
                "x_tail": xt,
                "lhsT_full": lhsT_full,
                "lhsT_tail": lhsT_tail,
                "sconst": sconst,
            }
        )

    res = run_bass_kernel_spmd(nc, in_maps, core_ids=list(range(NCORES)), trace=TRACE)
    kernel.last_exec_time_ns = res.exec_time_ns
    kernel.last_result = res
    out = np.empty((B, O, T), np.float32)
    for c in range(NCORES):
        out[c * NB : (c + 1) * NB] = res.results[c]["y"].astype(np.float32).reshape(
            NB, O, T
        )
    return np.ascontiguousarray(out.transpose(0, 2, 1))


kernel.last_exec_time_ns = None
kernel.last_result = None
